# revision 1
# baseline (speedup 1.0000x reference)
"""Trainium2 Bass kernel for ChaoticAttentionLayer.

Math (reference):
    q = r_s * sig(zq) * (1 - sig(zq)),  zq = query @ Wq.T + bq,  r_s = 4*sigmoid(r)
    k likewise, v = value @ Wv.T + bv
    out = softmax(q k^T / 8) v @ Wo.T + bo   (per head, D=64)

Device decomposition:
    g = sig*(1-sig); scores = (r_s^2/8) * g(zq) . g(zk); the r_s^2/8 factor is
    folded into the Exp activation's scale. Scores are bounded in [0, 8] for
    any r, so softmax runs max-free: exp(scores) directly.

Sharding: 8 cores = 4 batches x 2 head-groups (4 heads each). Each core
computes partial out[b] = attn_hg @ Wo[:, hg].T; host sums the two partials
per batch and adds bo.

Structure (per core, all bf16 on matmul paths):
  - Scores are computed transposed, S^T[s_block, t], two heads per Exp call
    (row-tiled 64-contraction matmuls run concurrently on the PE).
  - attn@V uses V as the stationary operand: av[d, t] = V'^T ex, one
    N=512 matmul per (head, s_block). V' is padded per head to 128 cols:
    even head [V | 1 | 0*63], odd head [0*63 | 1 | V], so the even head's
    numerator lands at PSUM partitions 0..63 and the odd head's at 64..127.
    The softmax denominator comes out at row 64 / 63 respectively.
  - Normalization: 1/den via reciprocal_approx_fast on the den row, DMA
    partition-broadcast of the reciprocal row, then one tensor_mul per head
    writes the normalized, already-transposed attention output aTn[128, t]
    (both heads of a pair stacked) -- no PE transposes needed.
  - Out-projection: full K=128 contraction lhsT=aTn, accumulated over the
    two head pairs; final scale+copy and DMA out.
"""

import numpy as np
import ml_dtypes
from contextlib import ExitStack

try:
    import concourse.bass as bass
except ImportError:  # pragma: no cover
    import sys

    sys.path.insert(0, "/opt/trn_rl_repo")
    import concourse.bass as bass

import concourse.bacc as bacc
import concourse.tile as tile
from concourse import mybir
from concourse.bass_utils import run_bass_kernel_spmd

F32 = mybir.dt.float32
BF16 = mybir.dt.bfloat16
I16 = mybir.dt.int16
SCHRAUD_B = 16256.0 - 7.4        # bf16 Schraudolph bias (round-to-nearest)
DVE_SBS = frozenset({3, 5, 7, 9, 11, 13})  # exp chunks computed on DVE
AF = mybir.ActivationFunctionType
BF16NP = ml_dtypes.bfloat16

B, T, S, E, H = 4, 2048, 2048, 512, 8
D = E // H           # 64 head dim
HG = 2               # head-groups per batch (cores per batch)
HPG = H // HG        # 4 heads per group
EG = HPG * D         # 256 dims per head group
NCORES = 8
P = 128              # partitions
TCH = 512            # t-chunk (psum free dim)
NSB = S // P         # 16 s-blocks
NKT = E // P         # 4 contraction tiles of 128
NTC = T // TCH       # 4 t-chunks


def _build():
    nc = bacc.Bacc("TRN2", target_bir_lowering=False, debug=False,
                   num_devices=NCORES)

    xqT = nc.dram_tensor("xqT", [E, T], BF16, kind="ExternalInput")
    xkT = nc.dram_tensor("xkT", [E, S], BF16, kind="ExternalInput")
    xvT = nc.dram_tensor("xvT", [E + 1, S], BF16, kind="ExternalInput")
    wqT = nc.dram_tensor("wqT", [E, EG], BF16, kind="ExternalInput")
    wkT = nc.dram_tensor("wkT", [E, EG], BF16, kind="ExternalInput")
    wvT = nc.dram_tensor("wvT", [E + 1, EG], BF16, kind="ExternalInput")
    woT = nc.dram_tensor("woT", [EG, E], BF16, kind="ExternalInput")
    bq = nc.dram_tensor("bq", [EG, 1], F32, kind="ExternalInput")
    bk = nc.dram_tensor("bk", [EG, 1], F32, kind="ExternalInput")
    cexp = nc.dram_tensor("cexp", [1, 1], F32, kind="ExternalInput")
    cexp2 = nc.dram_tensor("cexp2", [1, 1], F32, kind="ExternalInput")
    out = nc.dram_tensor("out", [T, E], F32, kind="ExternalOutput")

    with tile.TileContext(nc) as tc, ExitStack() as ctx:
        persist = ctx.enter_context(tc.tile_pool(name="persist", bufs=1))

        # --- persistent SBUF state ---
        # K-projection inputs first: they gate the whole pipeline.
        wk_sb = []
        bk_sb = []
        for kt in range(NKT):
            tk = persist.tile([P, EG], BF16, tag=f"wk{kt}")
            nc.sync.dma_start(out=tk, in_=wkT[kt * P:(kt + 1) * P, :])
            wk_sb.append(tk)
        for c in range(EG // P):
            tb2 = persist.tile([P, 1], F32, tag=f"bk{c}")
            nc.sync.dma_start(out=tb2, in_=bk[c * P:(c + 1) * P, :])
            bk_sb.append(tb2)
        # projected tensors, resident for the whole kernel; chunked into
        # [P, TCH] column tiles so consumers unblock per-chunk.
        QT_sb = [[persist.tile([P, TCH], BF16, tag=f"qt{c}_{q}",
                               name=f"qt{c}_{q}") for q in range(NTC)]
                 for c in range(EG // P)]
        KT_sb = [[persist.tile([P, TCH], BF16, tag=f"kt{c}_{q}",
                               name=f"ktile{c}_{q}") for q in range(NTC)]
                 for c in range(EG // P)]
        # V', padded to 128 cols per head:
        #   even head h: [V_h (64) | ones (1) | zeros (63)]
        #   odd  head h: [ones (1) | zeros (63) | V_h (64)]
        V2_sb = [persist.tile([P, HPG, P], BF16, tag=f"v{sc}", name=f"v{sc}")
                 for sc in range(NSB)]

        # --- x inputs + remaining weights, in consumption order ---
        xk_sb = [[None] * NTC for _ in range(NKT)]
        xv_sb = [[None] * NTC for _ in range(NKT)]
        xq_sb = [[None] * NTC for _ in range(NKT)]

        def load_x(dst, src, kt, q):
            t = persist.tile([P, TCH], BF16, tag=f"{dst}{kt}_{q}",
                             name=f"{dst}{kt}_{q}")
            nc.sync.dma_start(
                out=t, in_=src[kt * P:(kt + 1) * P, q * TCH:(q + 1) * TCH])
            return t

        for kt in range(NKT):
            xk_sb[kt][0] = load_x("xk", xkT, kt, 0)

        wq_sb = []
        bq_sb = []
        for kt in range(NKT):
            tq = persist.tile([P, EG], BF16, tag=f"wq{kt}")
            nc.sync.dma_start(out=tq, in_=wqT[kt * P:(kt + 1) * P, :])
            wq_sb.append(tq)
        for c in range(EG // P):
            tb_ = persist.tile([P, 1], F32, tag=f"bq{c}")
            nc.sync.dma_start(out=tb_, in_=bq[c * P:(c + 1) * P, :])
            bq_sb.append(tb_)
        for kt in range(NKT):
            xq_sb[kt][0] = load_x("xq", xqT, kt, 0)

        wv_sb = []
        for kt in range(NKT):
            tv = persist.tile([P, EG], BF16, tag=f"wv{kt}")
            nc.sync.dma_start(out=tv, in_=wvT[kt * P:(kt + 1) * P, :])
            wv_sb.append(tv)
        wv4_sb = persist.tile([1, EG], BF16, tag="wv4")
        nc.sync.dma_start(out=wv4_sb, in_=wvT[E:E + 1, :])
        ones_sb = persist.tile([1, S], BF16, tag="ones")
        nc.sync.dma_start(out=ones_sb, in_=xvT[E:E + 1, :])
        for kt in range(NKT):
            xv_sb[kt][0] = load_x("xvr", xvT, kt, 0)

        ones64 = persist.tile([P, D], BF16, tag="ones64")
        nc.vector.memset(ones64, 1.0)

        cexp_sb = persist.tile([P, 1], F32, tag="cexp")
        cap = cexp[:, :]
        nc.sync.dma_start(
            out=cexp_sb,
            in_=bass.AP(tensor=cap.tensor, offset=cap.offset, ap=[[0, P], [1, 1]]),
        )
        cexp2_sb = persist.tile([P, 1], F32, tag="cexp2")
        cap2 = cexp2[:, :]
        nc.sync.dma_start(
            out=cexp2_sb,
            in_=bass.AP(tensor=cap2.tensor, offset=cap2.offset,
                        ap=[[0, P], [1, 1]]),
        )

        for q in range(1, NTC):
            for kt in range(NKT):
                xk_sb[kt][q] = load_x("xk", xkT, kt, q)
            for kt in range(NKT):
                xq_sb[kt][q] = load_x("xq", xqT, kt, q)
            for kt in range(NKT):
                xv_sb[kt][q] = load_x("xvr", xvT, kt, q)

        # out-proj weights are only needed much later.
        wo_sb = []
        for kb in range(EG // P):
            to = persist.tile([P, E], BF16, tag=f"wo{kb}")
            nc.sync.dma_start(out=to, in_=woT[kb * P:(kb + 1) * P, :])
            wo_sb.append(to)

        def qk_proj_chunk(pool, x_sb, w_sb, b_sb, out_tiles, c, tcq, sig):
            ps = pool.tile([P, TCH], F32, tag="ps", name=f"ps_{c}_{tcq}")
            for kt in range(NKT):
                nc.tensor.matmul(
                    ps, w_sb[kt][:, c * P:(c + 1) * P],
                    x_sb[kt][tcq],
                    start=(kt == 0), stop=(kt == NKT - 1))
            # sig'(z) = (1 - tanh^2(z/2)) / 4 -- tanh shares ACT's exp
            # table set, so the whole kernel needs one ACT_TABLE_LOAD.
            y = sig.tile([P, TCH], F32, tag="y", name=f"y_{c}_{tcq}")
            nc.scalar.activation(y, ps, AF.Tanh, bias=b_sb[c], scale=0.5)
            y2 = sig.tile([P, TCH], F32, tag="y2", name=f"y2_{c}_{tcq}")
            nc.vector.tensor_mul(y2, y, y)
            nc.vector.tensor_scalar(out_tiles[c][tcq], y2, -0.25, 0.25,
                                    mybir.AluOpType.mult,
                                    mybir.AluOpType.add)

        def proj_group(tcq, psp, sig):
            for c in range(EG // P):
                qk_proj_chunk(psp, xk_sb, wk_sb, bk_sb, KT_sb, c, tcq, sig)
            for c in range(EG // P):
                qk_proj_chunk(psp, xq_sb, wq_sb, bq_sb, QT_sb, c, tcq, sig)
            for sci in range(4):
                sc = tcq * 4 + sci
                ps = psp.tile([P, TCH], F32, tag="ps", name=f"psv_{sc}")
                for kt in range(NKT):
                    nc.tensor.matmul(ps[:, 0:EG],
                                     xv_sb[kt][tcq][:, sci * P:(sci + 1) * P],
                                     wv_sb[kt], start=(kt == 0), stop=False)
                nc.tensor.matmul(ps[:, 0:EG], ones_sb[:, sc * P:(sc + 1) * P],
                                 wv4_sb, start=False, stop=True)
                v2 = V2_sb[sc]
                psv = ps[:, 0:EG].rearrange("p (h d) -> p h d", h=HPG)
                # even heads: V at cols 0..63, ones at 64, zeros above
                nc.vector.tensor_copy(v2[:, 0::2, 0:D], psv[:, 0::2, :])
                nc.vector.memset(v2[:, 0::2, D:D + 1], 1.0)
                nc.vector.memset(v2[:, 0::2, D + 1:P], 0.0)
                # odd heads: ones at 0, zeros at 1..63, V at cols 64..127
                nc.vector.tensor_copy(v2[:, 1::2, D:P], psv[:, 1::2, :])
                nc.vector.memset(v2[:, 1::2, 0:1], 1.0)
                nc.vector.memset(v2[:, 1::2, 1:D], 0.0)

        # --- attention + out-projection ---
        expp = ctx.enter_context(tc.tile_pool(name="expp", bufs=3))
        aTnp = ctx.enter_context(tc.tile_pool(name="aTnp", bufs=2))
        avsp = ctx.enter_context(tc.tile_pool(name="avsp", bufs=2))
        rdbp = ctx.enter_context(tc.tile_pool(name="rdbp", bufs=2))
        outp = ctx.enter_context(tc.tile_pool(name="outp", bufs=3))

        # deferred-emission state: norm-chain PE ops and out-projection are
        # emitted a few s-blocks into the NEXT phase so the in-order PE queue
        # never stalls on them.
        state = {"norm": None, "out": None}

        def attention_tci(tci, pss, pmix):
            aTn = [aTnp.tile([P, TCH], BF16, tag=f"aTn{hp}",
                             name=f"aTn_{tci}_{hp}") for hp in range(2)]
            for hp in range(2):
                avE = pmix.tile([P, TCH], F32, tag="av0",
                                name=f"avE_{tci}_{hp}")
                avO = pmix.tile([P, TCH], F32, tag="av1",
                                name=f"avO_{tci}_{hp}")
                avs = [avE, avO]
                exs = [None] * NSB

                def emit_av(sb, avs=avs, exs=exs, hp=hp):
                    for hi in range(2):
                        h = 2 * hp + hi
                        nc.tensor.matmul(
                            avs[hi],
                            V2_sb[sb][:, h, :],
                            exs[sb][:, hi * TCH:(hi + 1) * TCH],
                            start=(sb == 0), stop=(sb == NSB - 1),
                            skip_group_check=(sb != 0))

                for sb in range(NSB):
                    ps = pss.tile([P, 2 * TCH], F32, tag="ps",
                                  name=f"ps_{tci}_{hp}_{sb}")
                    for hi in range(2):
                        h = 2 * hp + hi
                        off = hi * D
                        nc.tensor.matmul(
                            ps[:, hi * TCH:(hi + 1) * TCH],
                            KT_sb[hp][sb // 4][off:off + D,
                                               (sb % 4) * P:(sb % 4 + 1) * P],
                            QT_sb[hp][tci][off:off + D, :],
                            start=True, stop=True,
                            tile_position=(off, 0))
                    if sb in DVE_SBS:
                        # Schraudolph exp on the DVE: bf16 bits of 2^t as an
                        # int16 affine map of the raw scores.
                        exi = expp.tile([P, 2 * TCH], I16, tag="exi",
                                        name=f"exi_{tci}_{hp}_{sb}")
                        nc.vector.tensor_scalar(
                            exi, ps, cexp2_sb, SCHRAUD_B,
                            mybir.AluOpType.mult, mybir.AluOpType.add)
                        exs[sb] = exi.bitcast(BF16)
                    else:
                        ex = expp.tile([P, 2 * TCH], BF16, tag="ex",
                                       name=f"ex_{tci}_{hp}_{sb}")
                        nc.scalar.activation(ex, ps, AF.Exp, scale=cexp_sb)
                        exs[sb] = ex
                    if sb == 2 and state["norm"] is not None:
                        state["norm"]()
                        state["norm"] = None
                    if sb == 12 and hp == 0 and state["out"] is not None:
                        state["out"]()
                        state["out"] = None
                    if sb >= 1:
                        emit_av(sb - 1)
                        # tiny dependency-free filler matmul: keeps the PE
                        # activity monitor warm (K=8/8) across the exp-paced
                        # micro-stalls; writes garbage to the scratch bank.
                        fil = pmix.tile([P, TCH], F32, tag="rdn",
                                        name=f"fil_{tci}_{hp}_{sb}")
                        nc.tensor.matmul(fil[0:32, 0:64],
                                         wq_sb[0][:, 0:32],
                                         xq_sb[0][0][:, 0:64],
                                         start=True, stop=True)
                emit_av(NSB - 1)

                # copy both av banks to SBUF (bf16) immediately: frees the
                # PSUM banks for the next phase and provides SBUF operands
                # for the PE den-broadcast matmuls.
                avsE = avsp.tile([P, TCH], BF16, tag="avsE",
                                 name=f"avsE_{tci}_{hp}")
                avsO = avsp.tile([P, TCH], BF16, tag="avsO",
                                 name=f"avsO_{tci}_{hp}")
                nc.vector.tensor_copy(avsE[0:D + 1, :], avE[0:D + 1, :])
                nc.vector.tensor_copy(avsO, avO)

                def emit_norm(avsE=avsE, avsO=avsO, dst=aTn[hp], tci=tci,
                              hp=hp):
                    # broadcast each head's den row across 64 partitions via
                    # a K=1 outer-product matmul, reciprocal once, scale.
                    rdn = pmix.tile([P, TCH], F32, tag="rdn",
                                    name=f"rdn_{tci}_{hp}")
                    nc.tensor.matmul(rdn[0:D, :], ones64[D:D + 1, :],
                                     avsE[D:D + 1, :], start=True, stop=True)
                    nc.tensor.matmul(rdn[D:P, :], ones64[0:1, :],
                                     avsO[0:1, :], start=True, stop=True)
                    rdb = rdbp.tile([P, TCH], F32, tag="rdb",
                                    name=f"rdb_{tci}_{hp}")
                    nc.vector.reciprocal_approx_fast(out=rdb, in_=rdn)
                    nc.vector.tensor_mul(dst[0:D, :], avsE[0:D, :],
                                         rdb[0:D, :])
                    nc.vector.tensor_mul(dst[D:P, :], avsO[D:P, :],
                                         rdb[D:P, :])

                state["norm"] = emit_norm

            def emit_outproj(aTn=aTn, tci=tci):
                for ts in range(4):
                    pf = pmix.tile([P, E], F32, tag="pf",
                                   name=f"pf_{tci}_{ts}")
                    for hp2 in range(2):
                        nc.tensor.matmul(pf,
                                         aTn[hp2][:, ts * P:(ts + 1) * P],
                                         wo_sb[hp2],
                                         start=(hp2 == 0), stop=(hp2 == 1))
                    ot = outp.tile([P, E], F32, tag="ot",
                                   name=f"ot_{tci}_{ts}")
                    nc.vector.tensor_copy(ot, pf)
                    row = (tci * 4 + ts) * P
                    nc.sync.dma_start(out=out[row:row + P, :], in_=ot)

            state["out"] = emit_outproj

        with ExitStack() as cp:
            psp = cp.enter_context(
                tc.tile_pool(name="psp", bufs=2, space="PSUM"))
            sig = cp.enter_context(tc.tile_pool(name="sig", bufs=4))
            for tcq in range(NTC):
                proj_group(tcq, psp, sig)
        with ExitStack() as c4:
            pss = c4.enter_context(
                tc.tile_pool(name="pss", bufs=2, space="PSUM"))
            pmix = c4.enter_context(
                tc.tile_pool(name="pmix", bufs=1, space="PSUM"))
            for tcq in range(NTC):
                attention_tci(tcq, pss, pmix)
            state["norm"]()
            state["out"]()

    nc.compile()
    return nc


_NC = None
_LAST_IN_MAPS = None


def _get_nc():
    global _NC
    if _NC is None:
        _NC = _build()
    return _NC


def kernel(**inputs):
    query = np.asarray(inputs["query"], np.float32)
    key_ = np.asarray(inputs["key_"] if "key_" in inputs else inputs["key"],
                      np.float32)
    value = np.asarray(inputs["value"], np.float32)
    Wq = np.asarray(inputs["Wq"], np.float32)
    bq = np.asarray(inputs["bq"], np.float32)
    Wk = np.asarray(inputs["Wk"], np.float32)
    bk = np.asarray(inputs["bk"], np.float32)
    Wv = np.asarray(inputs["Wv"], np.float32)
    bv = np.asarray(inputs["bv"], np.float32)
    Wo = np.asarray(inputs["Wo"], np.float32)
    bo = np.asarray(inputs["bo"], np.float32)
    r = float(np.asarray(inputs["r"]).reshape(-1)[0])

    r_s = 4.0 / (1.0 + np.exp(-np.float64(r)))
    c = np.float32(r_s * r_s / 8.0)

    WqT = Wq.T.astype(BF16NP)
    WkT = Wk.T.astype(BF16NP)
    WoT = Wo.T.astype(BF16NP)
    WvTa = np.concatenate([Wv.T, bv[None, :]], axis=0).astype(BF16NP)

    in_maps = []
    for b in range(B):
        xqT = np.ascontiguousarray(query[b].T).astype(BF16NP)
        xkT = np.ascontiguousarray(key_[b].T).astype(BF16NP)
        xvT = np.concatenate(
            [np.ascontiguousarray(value[b].T), np.ones((1, S), np.float32)],
            axis=0).astype(BF16NP)
        for g in range(HG):
            cols = slice(g * EG, (g + 1) * EG)
            in_maps.append(dict(
                xqT=xqT, xkT=xkT, xvT=xvT,
                wqT=np.ascontiguousarray(WqT[:, cols]),
                wkT=np.ascontiguousarray(WkT[:, cols]),
                wvT=np.ascontiguousarray(WvTa[:, cols]),
                woT=np.ascontiguousarray(WoT[cols, :]),
                bq=np.ascontiguousarray(0.5 * bq[cols, None]),
                bk=np.ascontiguousarray(0.5 * bk[cols, None]),
                cexp=np.array([[c]], np.float32),
                cexp2=np.array([[c * 128.0 * np.log2(np.e)]], np.float32),
            ))

    global _LAST_IN_MAPS
    _LAST_IN_MAPS = in_maps
    res = run_bass_kernel_spmd(_get_nc(), in_maps, core_ids=list(range(NCORES)))
    out = np.empty((B, T, E), np.float32)
    for b in range(B):
        out[b] = res.results[HG * b]["out"]
        for g in range(1, HG):
            out[b] += res.results[HG * b + g]["out"]
        out[b] += bo[None, :]
    return out



# revision 7
# speedup vs baseline: 1.1451x; 1.1451x over previous
"""Trainium2 Bass kernel for ChaoticAttentionLayer.

Math (reference):
    q = r_s * sig(zq) * (1 - sig(zq)),  zq = query @ Wq.T + bq,  r_s = 4*sigmoid(r)
    k likewise, v = value @ Wv.T + bv
    out = softmax(q k^T / 8) v @ Wo.T + bo   (per head, D=64)

Device decomposition:
    g = sig*(1-sig); scores = (r_s^2/8) * g(zq) . g(zk); the r_s^2/8 factor is
    folded into the Exp activation's scale. Scores are bounded in [0, 8] for
    any r, so softmax runs max-free: exp(scores) directly.

Sharding: 8 cores = 4 batches x 2 head-groups (4 heads each). Each core
computes partial out[b] = attn_hg @ Wo[:, hg].T; host sums the two partials
per batch and adds bo.

Structure (per core, all bf16 on matmul paths):
  - Scores are computed transposed, S^T[s_block, t], two heads per block
    (row-tiled 64-contraction matmuls run concurrently on the PE).
  - exp alternates between ACT (table exp, odd s-blocks) and DVE
    (Schraudolph int16 affine, even s-blocks) so both engines pipeline at
    the PE pace.
  - attn@V uses V as the stationary operand: av[d, t] = V'^T ex, one
    N=512 matmul per (head, s_block). V' is padded per head to 128 cols:
    even head [V | 1 | fill], odd head [1 | fill | V] (fill cols produce
    PSUM rows that are never read; tiles are memset to 1.0 once).
  - Normalization: den rows broadcast across partitions via K=1 matmuls,
    reciprocal_approx_fast, bf16 cast, then GPSIMD tensor_muls write the
    normalized transposed attention output aTn[128, t] (GPSIMD is
    otherwise idle; this keeps DVE free for exp).
  - Out-projection: full K=128 contraction lhsT=aTn, accumulated over the
    two head pairs; double-buffered PSUM, copies split ACT/DVE.
  - Input DMAs are split across the two HWDGE engines (Sync + Scalar) with
    >=2KB per-partition lines; x tensors load as [128, 1024] halves.
  - Q projection for t-chunks 1..3 is deferred into the attention phase so
    attention starts as soon as K/V/Q0 are projected.
"""

import numpy as np
import ml_dtypes
from contextlib import ExitStack

try:
    import concourse.bass as bass
except ImportError:  # pragma: no cover
    import sys

    sys.path.insert(0, "/opt/trn_rl_repo")
    import concourse.bass as bass

import concourse.bacc as bacc
import concourse.tile as tile
from concourse import mybir
from concourse.bass_utils import run_bass_kernel_spmd

F32 = mybir.dt.float32
BF16 = mybir.dt.bfloat16
I16 = mybir.dt.int16
SCHRAUD_B = 16256.0 - 7.4        # bf16 Schraudolph bias (round-to-nearest)
ACT_SBS = frozenset({1, 3, 5, 7, 9, 11, 13, 15})  # exp chunks on ACT
AF = mybir.ActivationFunctionType
BF16NP = ml_dtypes.bfloat16

B, T, S, E, H = 4, 2048, 2048, 512, 8
D = E // H           # 64 head dim
HG = 2               # head-groups per batch (cores per batch)
HPG = H // HG        # 4 heads per group
EG = HPG * D         # 256 dims per head group
NCORES = 8
P = 128              # partitions
TCH = 512            # t-chunk (psum free dim)
NSB = S // P         # 16 s-blocks
NKT = E // P         # 4 contraction tiles of 128
NTC = T // TCH       # 4 t-chunks


def _build():
    nc = bacc.Bacc("TRN2", target_bir_lowering=False, debug=False,
                   num_devices=NCORES)

    xqT = nc.dram_tensor("xqT", [E, T], BF16, kind="ExternalInput")
    xkT = nc.dram_tensor("xkT", [E, S], BF16, kind="ExternalInput")
    xvT = nc.dram_tensor("xvT", [E + 1, S], BF16, kind="ExternalInput")
    wpackT = nc.dram_tensor("wpackT", [E, 3 * EG], BF16, kind="ExternalInput")
    wv4 = nc.dram_tensor("wv4", [1, EG], BF16, kind="ExternalInput")
    woT = nc.dram_tensor("woT", [EG, E], BF16, kind="ExternalInput")
    bpack = nc.dram_tensor("bpack", [P, 4], F32, kind="ExternalInput")
    cexp = nc.dram_tensor("cexp", [1, 1], F32, kind="ExternalInput")
    cexp2 = nc.dram_tensor("cexp2", [1, 1], F32, kind="ExternalInput")
    out = nc.dram_tensor("out", [T, E], F32, kind="ExternalOutput")

    with tile.TileContext(nc) as tc, ExitStack() as ctx:
        persist = ctx.enter_context(tc.tile_pool(name="persist", bufs=1))

        # --- input loads: the Sync HWDGE queue carries the big x tensors in
        # consumption order; the Scalar (ACT) HWDGE queue carries weights and
        # biases in parallel.
        xk_sb = [[None] * 2 for _ in range(NKT)]
        for h2 in range(2):
            for kt in range(NKT):
                t_ = persist.tile([P, 2 * TCH], BF16, tag=f"xk{kt}_{h2}",
                                  name=f"xk{kt}_{h2}")
                nc.sync.dma_start(
                    out=t_,
                    in_=xkT[kt * P:(kt + 1) * P,
                            h2 * 2 * TCH:(h2 + 1) * 2 * TCH])
                xk_sb[kt][h2] = t_

        wpk_sb = []
        for kt in range(NKT):
            tw = persist.tile([P, 3 * EG], BF16, tag=f"wpk{kt}")
            nc.scalar.dma_start(out=tw, in_=wpackT[kt * P:(kt + 1) * P, :])
            wpk_sb.append(tw)
        wq_sb = [tw[:, 0:EG] for tw in wpk_sb]
        wk_sb = [tw[:, EG:2 * EG] for tw in wpk_sb]
        wv_sb = [tw[:, 2 * EG:3 * EG] for tw in wpk_sb]

        bp_sb = persist.tile([P, 4], F32, tag="bp")
        nc.scalar.dma_start(out=bp_sb, in_=bpack[:, :])
        bq_sb = [bp_sb[:, c:c + 1] for c in range(2)]
        bk_sb = [bp_sb[:, 2 + c:3 + c] for c in range(2)]

        cexp_sb = persist.tile([P, 1], F32, tag="cexp")
        cap = cexp[:, :]
        nc.scalar.dma_start(
            out=cexp_sb,
            in_=bass.AP(tensor=cap.tensor, offset=cap.offset, ap=[[0, P], [1, 1]]),
        )
        cexp2_sb = persist.tile([P, 1], F32, tag="cexp2")
        cap2 = cexp2[:, :]
        nc.scalar.dma_start(
            out=cexp2_sb,
            in_=bass.AP(tensor=cap2.tensor, offset=cap2.offset,
                        ap=[[0, P], [1, 1]]),
        )

        # xv first half interleaved ahead of xq0 so V-projection is fed early
        xv_sb = [[None] * 2 for _ in range(NKT)]
        for kt in range(NKT):
            tv = persist.tile([P, 2 * TCH], BF16, tag=f"xv{kt}_0",
                              name=f"xv{kt}_0")
            nc.sync.dma_start(out=tv, in_=xvT[kt * P:(kt + 1) * P, 0:2 * TCH])
            xv_sb[kt][0] = tv

        xq0_sb = []
        for kt in range(NKT):
            tq = persist.tile([P, TCH], BF16, tag=f"xq0_{kt}",
                              name=f"xq0_{kt}")
            nc.sync.dma_start(out=tq, in_=xqT[kt * P:(kt + 1) * P, 0:TCH])
            xq0_sb.append(tq)

        for kt in range(NKT):
            tv = persist.tile([P, 2 * TCH], BF16, tag=f"xv{kt}_1",
                              name=f"xv{kt}_1")
            nc.sync.dma_start(out=tv,
                              in_=xvT[kt * P:(kt + 1) * P, 2 * TCH:S])
            xv_sb[kt][1] = tv

        wv4_sb = persist.tile([1, EG], BF16, tag="wv4")
        nc.scalar.dma_start(out=wv4_sb, in_=wv4[:, :])
        ones_sb = persist.tile([1, S], BF16, tag="ones")
        nc.scalar.dma_start(out=ones_sb, in_=xvT[E:E + 1, :])
        wo_sb = []
        for kb in range(EG // P):
            to = persist.tile([P, E], BF16, tag=f"wo{kb}")
            nc.scalar.dma_start(out=to, in_=woT[kb * P:(kb + 1) * P, :])
            wo_sb.append(to)

        # q chunks 1..3, needed only once attention is underway
        xqr_sb = []
        for kt in range(NKT):
            tr = persist.tile([P, 3 * TCH], BF16, tag=f"xqr{kt}",
                              name=f"xqr{kt}")
            nc.sync.dma_start(out=tr, in_=xqT[kt * P:(kt + 1) * P, TCH:T])
            xqr_sb.append(tr)

        ones64 = persist.tile([P, D], BF16, tag="ones64")
        nc.vector.memset(ones64, 1.0)

        # projected tensors, resident for the whole kernel
        QT_sb = [[persist.tile([P, TCH], BF16, tag=f"qt{c}_{q}",
                               name=f"qt{c}_{q}") for q in range(NTC)]
                 for c in range(EG // P)]
        KT_sb = [[persist.tile([P, TCH], BF16, tag=f"kt{c}_{q}",
                               name=f"ktile{c}_{q}") for q in range(NTC)]
                 for c in range(EG // P)]
        # V', padded to 128 cols per head:
        #   even head h: [V_h (64) | 1.0 fill (64)]  (den col at 64)
        #   odd  head h: [1.0 fill (64) | V_h (64)]  (den col at 0)
        # fill cols produce av rows that are never read.
        V2_sb = [persist.tile([P, HPG, P], BF16, tag=f"v{sc}", name=f"v{sc}")
                 for sc in range(NSB)]
        for sc in range(NSB):
            nc.vector.memset(V2_sb[sc], 1.0)

        def qk_proj_chunk(psp, sigp, x_of_kt, w_sb, b_sb, out_tiles, c, tcq,
                          psum_shape=None, psum_tag="ps"):
            ps = psp.tile(psum_shape or [P, TCH], F32, tag=psum_tag,
                          name=f"ps_{id(out_tiles)}_{c}_{tcq}")
            pz = ps[:, 0:TCH]
            for kt in range(NKT):
                nc.tensor.matmul(
                    pz, w_sb[kt][:, c * P:(c + 1) * P],
                    x_of_kt(kt),
                    start=(kt == 0), stop=(kt == NKT - 1))
            # sig'(z) = (1 - tanh^2(z/2)) / 4 -- tanh shares ACT's exp
            # table set, so the whole kernel needs one ACT_TABLE_LOAD.
            y = sigp.tile([P, TCH], F32, tag="y", name=f"y_{c}_{tcq}")
            nc.scalar.activation(y, pz, AF.Tanh, bias=b_sb[c], scale=0.5)
            y2 = sigp.tile([P, TCH], F32, tag="y2", name=f"y2_{c}_{tcq}")
            nc.gpsimd.tensor_mul(y2, y, y)
            nc.vector.tensor_scalar(out_tiles[c][tcq], y2, -0.25, 0.25,
                                    mybir.AluOpType.mult,
                                    mybir.AluOpType.add)

        # --- phase 1: K projection (all t), Q chunk 0, V projection ---
        with ExitStack() as cp:
            psp = cp.enter_context(
                tc.tile_pool(name="psp", bufs=2, space="PSUM"))
            sig = cp.enter_context(tc.tile_pool(name="sig", bufs=4))
            for tcq in range(NTC):
                for c in range(EG // P):
                    xslice = (lambda kt, tcq=tcq:
                              xk_sb[kt][tcq // 2][:, (tcq % 2) * TCH:
                                                  (tcq % 2 + 1) * TCH])
                    qk_proj_chunk(psp, sig, xslice, wk_sb, bk_sb, KT_sb,
                                  c, tcq)
            for c in range(EG // P):
                qk_proj_chunk(psp, sig, lambda kt: xq0_sb[kt], wq_sb, bq_sb,
                              QT_sb, c, 0)
            for tcq in range(NTC):
                for sci in range(4):
                    sc = tcq * 4 + sci
                    ps = psp.tile([P, TCH], F32, tag="ps", name=f"psv_{sc}")
                    for kt in range(NKT):
                        nc.tensor.matmul(
                            ps[:, 0:EG],
                            xv_sb[kt][tcq // 2][:, (tcq % 2) * TCH + sci * P:
                                                (tcq % 2) * TCH + (sci + 1) * P],
                            wv_sb[kt], start=(kt == 0), stop=False)
                    nc.tensor.matmul(ps[:, 0:EG],
                                     ones_sb[:, sc * P:(sc + 1) * P],
                                     wv4_sb, start=False, stop=True)
                    v2 = V2_sb[sc]
                    psv = ps[:, 0:EG].rearrange("p (h d) -> p h d", h=HPG)
                    nc.vector.tensor_copy(v2[:, 0::2, 0:D], psv[:, 0::2, :])
                    nc.vector.tensor_copy(v2[:, 1::2, D:P], psv[:, 1::2, :])

        # --- attention + out-projection ---
        expp = ctx.enter_context(tc.tile_pool(name="expp", bufs=3))
        aTnp = ctx.enter_context(tc.tile_pool(name="aTnp", bufs=2))
        avsp = ctx.enter_context(tc.tile_pool(name="avsp", bufs=2))
        rdbp = ctx.enter_context(tc.tile_pool(name="rdbp", bufs=2))
        outp = ctx.enter_context(tc.tile_pool(name="outp", bufs=3))
        sig2 = ctx.enter_context(tc.tile_pool(name="sig2", bufs=2))

        # deferred-emission state: norm-chain PE ops and out-projection are
        # emitted a few s-blocks into the NEXT phase so the in-order PE queue
        # never stalls on them.
        state = {"norm": None, "out": None}

        def attention_tci(tci, pss, pmix, pfp):
            def qproj_deferred(tci2, c):
                xslice = (lambda kt, tci2=tci2:
                          xqr_sb[kt][:, (tci2 - 1) * TCH:tci2 * TCH])
                qk_proj_chunk(pfp, sig2, xslice, wq_sb, bq_sb, QT_sb,
                              c, tci2, psum_shape=[P, E], psum_tag="pf")

            aTn = [aTnp.tile([P, TCH], BF16, tag=f"aTn{hp}",
                             name=f"aTn_{tci}_{hp}") for hp in range(2)]
            for hp in range(2):
                avE = pmix.tile([P, TCH], F32, tag="av0",
                                name=f"avE_{tci}_{hp}")
                avO = pmix.tile([P, TCH], F32, tag="av1",
                                name=f"avO_{tci}_{hp}")
                avs = [avE, avO]
                exs = [None] * NSB

                def emit_av(sb, avs=avs, exs=exs, hp=hp):
                    for hi in range(2):
                        h = 2 * hp + hi
                        nc.tensor.matmul(
                            avs[hi],
                            V2_sb[sb][:, h, :],
                            exs[sb][:, hi * TCH:(hi + 1) * TCH],
                            start=(sb == 0), stop=(sb == NSB - 1),
                            skip_group_check=(sb != 0))

                for sb in range(NSB):
                    ps = pss.tile([P, 2 * TCH], F32, tag="ps",
                                  name=f"ps_{tci}_{hp}_{sb}")
                    for hi in range(2):
                        h = 2 * hp + hi
                        off = hi * D
                        nc.tensor.matmul(
                            ps[:, hi * TCH:(hi + 1) * TCH],
                            KT_sb[hp][sb // 4][off:off + D,
                                               (sb % 4) * P:(sb % 4 + 1) * P],
                            QT_sb[hp][tci][off:off + D, :],
                            start=True, stop=True,
                            tile_position=(off, 0))
                    if sb in ACT_SBS:
                        ex = expp.tile([P, 2 * TCH], BF16, tag="ex",
                                       name=f"ex_{tci}_{hp}_{sb}")
                        nc.scalar.activation(ex, ps, AF.Exp, scale=cexp_sb)
                        exs[sb] = ex
                    else:
                        # Schraudolph exp on the DVE: bf16 bits of 2^t as an
                        # int16 affine map of the raw scores.
                        exi = expp.tile([P, 2 * TCH], I16, tag="exi",
                                        name=f"exi_{tci}_{hp}_{sb}")
                        nc.vector.tensor_scalar(
                            exi, ps, cexp2_sb, SCHRAUD_B,
                            mybir.AluOpType.mult, mybir.AluOpType.add)
                        exs[sb] = exi.bitcast(BF16)
                    if sb == 2 and state["norm"] is not None:
                        state["norm"]()
                        state["norm"] = None
                    if sb == 12 and hp == 0 and state["out"] is not None:
                        state["out"]()
                        state["out"] = None
                    if hp == 1 and tci < NTC - 1 and sb in (5, 9):
                        qproj_deferred(tci + 1, 0 if sb == 5 else 1)
                    if sb >= 1:
                        emit_av(sb - 1)
                emit_av(NSB - 1)

                # copy both av banks to SBUF (bf16) on ACT immediately:
                # frees the PSUM banks and provides SBUF operands for the
                # PE den-broadcast matmuls.
                avsE = avsp.tile([P, TCH], BF16, tag="avsE",
                                 name=f"avsE_{tci}_{hp}")
                avsO = avsp.tile([P, TCH], BF16, tag="avsO",
                                 name=f"avsO_{tci}_{hp}")
                nc.scalar.copy(avsE[0:D + 1, :], avE[0:D + 1, :])
                nc.scalar.copy(avsO, avO)

                def emit_norm(avsE=avsE, avsO=avsO, dst=aTn[hp], tci=tci,
                              hp=hp, last=False):
                    # broadcast each head's den row across 64 partitions via
                    # a K=1 outer-product matmul, reciprocal once, cast to
                    # bf16, then tensor_muls on the (otherwise idle) GPSIMD.
                    rdn = pfp.tile([P, E], F32, tag="pf",
                                   name=f"rdn_{tci}_{hp}")[:, 0:TCH]
                    nc.tensor.matmul(rdn[0:D, :], ones64[D:D + 1, :],
                                     avsE[D:D + 1, :], start=True, stop=True)
                    nc.tensor.matmul(rdn[D:P, :], ones64[0:1, :],
                                     avsO[0:1, :], start=True, stop=True)
                    rdb = rdbp.tile([P, TCH], F32, tag="rdb",
                                    name=f"rdb_{tci}_{hp}")
                    nc.vector.reciprocal_approx_fast(out=rdb, in_=rdn)
                    rdbb = rdbp.tile([P, TCH], BF16, tag="rdbb",
                                     name=f"rdbb_{tci}_{hp}")
                    nc.vector.tensor_copy(rdbb, rdb)
                    eng = nc.vector if last else nc.gpsimd
                    eng.tensor_mul(dst[0:D, :], avsE[0:D, :], rdbb[0:D, :])
                    eng.tensor_mul(dst[D:P, :], avsO[D:P, :], rdbb[D:P, :])

                state["norm"] = emit_norm

            def emit_outproj(aTn=aTn, tci=tci):
                for ts in range(4):
                    pf = pfp.tile([P, E], F32, tag="pf",
                                  name=f"pf_{tci}_{ts}")
                    for hp2 in range(2):
                        nc.tensor.matmul(pf,
                                         aTn[hp2][:, ts * P:(ts + 1) * P],
                                         wo_sb[hp2],
                                         start=(hp2 == 0), stop=(hp2 == 1))
                    ot = outp.tile([P, E], F32, tag="ot",
                                   name=f"ot_{tci}_{ts}")
                    if ts == 3:
                        nc.vector.tensor_copy(ot, pf)
                    else:
                        nc.scalar.copy(ot, pf)
                    row = (tci * 4 + ts) * P
                    nc.sync.dma_start(out=out[row:row + P, :], in_=ot)

            state["out"] = emit_outproj

        with ExitStack() as c4:
            pss = c4.enter_context(
                tc.tile_pool(name="pss", bufs=2, space="PSUM"))
            pmix = c4.enter_context(
                tc.tile_pool(name="pmix", bufs=1, space="PSUM"))
            pfp = c4.enter_context(
                tc.tile_pool(name="pfp", bufs=2, space="PSUM"))
            for tcq in range(NTC):
                attention_tci(tcq, pss, pmix, pfp)
            state["norm"](last=True)
            state["out"]()

    nc.compile()
    return nc


_NC = None
_LAST_IN_MAPS = None


def _get_nc():
    global _NC
    if _NC is None:
        _NC = _build()
    return _NC


def kernel(**inputs):
    query = np.asarray(inputs["query"], np.float32)
    key_ = np.asarray(inputs["key_"] if "key_" in inputs else inputs["key"],
                      np.float32)
    value = np.asarray(inputs["value"], np.float32)
    Wq = np.asarray(inputs["Wq"], np.float32)
    bq = np.asarray(inputs["bq"], np.float32)
    Wk = np.asarray(inputs["Wk"], np.float32)
    bk = np.asarray(inputs["bk"], np.float32)
    Wv = np.asarray(inputs["Wv"], np.float32)
    bv = np.asarray(inputs["bv"], np.float32)
    Wo = np.asarray(inputs["Wo"], np.float32)
    bo = np.asarray(inputs["bo"], np.float32)
    r = float(np.asarray(inputs["r"]).reshape(-1)[0])

    r_s = 4.0 / (1.0 + np.exp(-np.float64(r)))
    c = np.float32(r_s * r_s / 8.0)

    WqT = Wq.T.astype(np.float32)
    WkT = Wk.T.astype(np.float32)
    WvT = Wv.T.astype(np.float32)
    WoT = Wo.T.astype(BF16NP)

    in_maps = []
    for b in range(B):
        xqT = np.ascontiguousarray(query[b].T).astype(BF16NP)
        xkT = np.ascontiguousarray(key_[b].T).astype(BF16NP)
        xvT = np.concatenate(
            [np.ascontiguousarray(value[b].T), np.ones((1, S), np.float32)],
            axis=0).astype(BF16NP)
        for g in range(HG):
            cols = slice(g * EG, (g + 1) * EG)
            wpack = np.concatenate(
                [WqT[:, cols], WkT[:, cols], WvT[:, cols]],
                axis=1).astype(BF16NP)
            bqh = 0.5 * bq[cols]
            bkh = 0.5 * bk[cols]
            bpk = np.stack([bqh[0:P], bqh[P:2 * P], bkh[0:P], bkh[P:2 * P]],
                           axis=1).astype(np.float32)
            in_maps.append(dict(
                xqT=xqT, xkT=xkT, xvT=xvT,
                wpackT=np.ascontiguousarray(wpack),
                wv4=np.ascontiguousarray(bv[None, cols]).astype(BF16NP),
                woT=np.ascontiguousarray(WoT[cols, :]),
                bpack=np.ascontiguousarray(bpk),
                cexp=np.array([[c]], np.float32),
                cexp2=np.array([[c * 128.0 * np.log2(np.e)]], np.float32),
            ))

    global _LAST_IN_MAPS
    _LAST_IN_MAPS = in_maps
    res = run_bass_kernel_spmd(_get_nc(), in_maps, core_ids=list(range(NCORES)))
    out = np.empty((B, T, E), np.float32)
    for b in range(B):
        out[b] = res.results[HG * b]["out"]
        for g in range(1, HG):
            out[b] += res.results[HG * b + g]["out"]
        out[b] += bo[None, :]
    return out
